# revision 9
# baseline (speedup 1.0000x reference)
"""Trainium2 Bass kernel for nn_MixquantLinear: O = ((dequant4(V) * S) @ dequant4(U)).T.

Output O is [4096, 4096] fp32, built purely from the GPTQ-quantized weights
(the activation input `x` is dead code in the reference and never touches the
device). Sharding: 4 slices over output rows (o) x 2 over output cols (i)
-> 8 cores, no collectives; host concatenates the blocks.

Per core:
  - unpack 4-bit nibbles (vector engine shift+mask on int32 words)
  - dequant affine (q - (z+1)) * scale, S folded into V's scale; chunks split
    between vector and scalar engines (per-partition scalar operands)
  - U slice PE-transposed into [rank, out] lhsT: 4 transposes per PSUM tile at
    bank-aligned offsets + one strided copy (avoids same-bank serialization)
  - fp16 matmuls (k-tiles of 128, N=512) accumulating fp32 in PSUM
  - strip-pipelined: strip-1 dequant interleaved with strip-0 matmul waves
Host-side work is layout-only (slicing/transposing packed int32 words and fp32
scale tables, concatenating outputs).
"""

import numpy as np

import concourse.bass as bass
import concourse.mybir as mybir
import concourse.tile as tile
from concourse import bacc
from concourse.bass_utils import run_bass_kernel_spmd
from concourse.masks import make_identity

IN_SIZE = 4096
OUT_SIZE = 4096
RANK = 1024
GROUPSIZE = 128
PACK = 8
P_O = 4
P_I = 2
O_SL = OUT_SIZE // P_O    # 1024
I_SL = IN_SIZE // P_I     # 2048
N_CORES = P_O * P_I
KT = RANK // 128          # 8
RT = KT
OT = O_SL // 128          # 8
N_STRIPS = 2
STRIP = I_SL // N_STRIPS  # 1024
GV = I_SL // GROUPSIZE    # 16
GU = RANK // GROUPSIZE    # 8

F16 = mybir.dt.float16
F32 = mybir.dt.float32
I32 = mybir.dt.int32
Alu = mybir.AluOpType
Act = mybir.ActivationFunctionType

_NC_CACHE = None
TRACE = False
LAST_RESULTS = None

V_DVE, V_ACT = 3, 2   # V-affine chunk split DVE:ACT
U_DVE, U_ACT = 1, 3   # U-affine chunk split (DVE busy unpacking during U phase)


def _build_nc():
    nc = bacc.Bacc("TRN2", target_bir_lowering=False)

    qvt = nc.dram_tensor("qvt", [128, RT * (I_SL // PACK)], I32, kind="ExternalInput")
    svt = nc.dram_tensor("svt", [128, RT * GV], F32, kind="ExternalInput")
    qzv = nc.dram_tensor("qzv", [GV, RANK // PACK], I32, kind="ExternalInput")
    qut = nc.dram_tensor("qut", [128, OT * (RANK // PACK)], I32, kind="ExternalInput")
    sut = nc.dram_tensor("sut", [128, OT * GU], F32, kind="ExternalInput")
    qzu = nc.dram_tensor("qzu", [GU, O_SL // PACK], I32, kind="ExternalInput")
    s_in = nc.dram_tensor("s", [128, RT], F32, kind="ExternalInput")
    out = nc.dram_tensor("out", [O_SL, I_SL], F32, kind="ExternalOutput")

    cnt = {"v": 0, "u": 0, "cp": 0}

    def affine(phase, out_ap, in_ap, zeff_col, a_col, bvn_col):
        """out = (in - zeff) * a; weighted split between DVE and ACT."""
        dve_w, act_w = (V_DVE, V_ACT) if phase == "v" else (U_DVE, U_ACT)
        i = cnt[phase]
        cnt[phase] += 1
        if i % (dve_w + act_w) < dve_w:
            nc.vector.tensor_scalar(
                out=out_ap, in0=in_ap, scalar1=zeff_col, scalar2=a_col,
                op0=Alu.subtract, op1=Alu.mult)
        else:
            nc.scalar.activation(out_ap, in_ap, Act.Identity, bias=bvn_col, scale=a_col)

    def copy_alt(out_ap, in_ap):
        cnt["cp"] += 1
        if cnt["cp"] % 2 == 0:
            nc.scalar.copy(out_ap, in_ap)
        else:
            nc.vector.tensor_copy(out_ap, in_ap)

    with tile.TileContext(nc) as tc:
        with (
            tc.tile_pool(name="const", bufs=1) as cp,
            tc.tile_pool(name="nibs", bufs=3) as nibp,
            tc.tile_pool(name="outsb", bufs=6) as outp,
        ):
            qvt_sb = cp.tile([128, RT * (I_SL // PACK)], I32, tag="qvt")
            qut_sb = cp.tile([128, OT * (RANK // PACK)], I32, tag="qut")
            svt_sb = cp.tile([128, RT * GV], F32, tag="svt")
            sut_sb = cp.tile([128, OT * GU], F32, tag="sut")
            s_sb = cp.tile([128, RT], F32, tag="s")
            qzv_sb = cp.tile([GV, RANK // PACK], I32, tag="qzv")
            qzu_sb = cp.tile([GU, O_SL // PACK], I32, tag="qzu")
            zv_unp = cp.tile([GV, RANK], I32, tag="zvu")
            zu_unp = cp.tile([GU, O_SL], I32, tag="zuu")
            zv_f = cp.tile([GV, RANK], F32, tag="zvf")
            zu_f = cp.tile([GU, O_SL], F32, tag="zuf")
            zeffv = cp.tile([128, RT * GV], F32, tag="zeffv")
            zeffu = cp.tile([128, OT * GU], F32, tag="zeffu")
            av = cp.tile([128, RT * GV], F32, tag="av")
            bvnv = cp.tile([128, RT * GV], F32, tag="bvnv")
            bvnu = cp.tile([128, OT * GU], F32, tag="bvnu")
            id16 = cp.tile([128, 128], F16, tag="id16")
            id32 = cp.tile([128, 128], F32, tag="id32")
            wut = cp.tile([128, OT * RANK], F16, tag="wut")
            lhsT = cp.tile([128, KT * O_SL], F16, tag="lhsT")
            rhs = [cp.tile([128, RT * STRIP], F16, tag=f"rhs{s}", name=f"rhs{s}")
                   for s in range(N_STRIPS)]

            # identities first (gpsimd), then input DMAs (zeros first: they gate
            # the small-table chain)
            make_identity(nc, id16[:])
            make_identity(nc, id32[:])
            nc.sync.dma_start(out=qzv_sb[:], in_=qzv[:])
            nc.sync.dma_start(out=qzu_sb[:], in_=qzu[:])
            nc.sync.dma_start(out=svt_sb[:], in_=svt[:])
            nc.sync.dma_start(out=sut_sb[:], in_=sut[:])
            nc.sync.dma_start(out=s_sb[:], in_=s_in[:])
            nc.sync.dma_start(out=qut_sb[:], in_=qut[:])
            nc.sync.dma_start(out=qvt_sb[:], in_=qvt[:])

            # ---- zeros unpack on DVE (tiny); rest of the table chain on ACT/PE ----
            zvu_r = zv_unp[:].rearrange("p (w j) -> p w j", j=PACK)
            zuu_r = zu_unp[:].rearrange("p (w j) -> p w j", j=PACK)
            for j in range(PACK):
                nc.vector.tensor_scalar(
                    out=zvu_r[:, :, j], in0=qzv_sb[:], scalar1=4 * j, scalar2=15,
                    op0=Alu.logical_shift_right, op1=Alu.bitwise_and)
                nc.vector.tensor_scalar(
                    out=zuu_r[:, :, j], in0=qzu_sb[:], scalar1=4 * j, scalar2=15,
                    op0=Alu.logical_shift_right, op1=Alu.bitwise_and)
            nc.scalar.copy(zv_f[:], zv_unp[:])
            nc.scalar.copy(zu_f[:], zu_unp[:])

            with tc.tile_pool(name="zps", bufs=2, space="PSUM") as zps:
                for t in range(RT):
                    pt = zps.tile([128, GV], F32, tag="zp", name="zp")
                    nc.tensor.transpose(pt[:], zv_f[:, t * 128:(t + 1) * 128],
                                        id32[:GV, :GV])
                    nc.scalar.add(zeffv[:, t * GV:(t + 1) * GV], pt[:], 1.0)
                for t in range(OT):
                    pt = zps.tile([128, GU], F32, tag="zp", name="zp")
                    nc.tensor.transpose(pt[:], zu_f[:, t * 128:(t + 1) * 128],
                                        id32[:GU, :GU])
                    nc.scalar.add(zeffu[:, t * GU:(t + 1) * GU], pt[:], 1.0)
            for t in range(RT):
                nc.scalar.mul(av[:, t * GV:(t + 1) * GV],
                              svt_sb[:, t * GV:(t + 1) * GV], s_sb[:, t:t + 1])

            # ---- U side: unpack (DVE), affines (mostly ACT), batched transposes ----
            def u_unpack(t):
                nibu = nibp.tile([128, RANK], I32, tag="nibu", name="nibu", bufs=8)
                nibu_r = nibu[:].rearrange("p (w j) -> p w j", j=PACK)
                words = qut_sb[:, t * (RANK // PACK):(t + 1) * (RANK // PACK)]
                for j in range(PACK):
                    nc.vector.tensor_scalar(
                        out=nibu_r[:, :, j], in0=words, scalar1=4 * j, scalar2=15,
                        op0=Alu.logical_shift_right, op1=Alu.bitwise_and)
                return nibu

            nibus = []
            for t in range(4):
                nibus.append(u_unpack(t))
            # bvn tables (DVE tensor_tensor) slotted here: zeros chain done by now
            nc.vector.tensor_tensor(bvnv[:], zeffv[:], av[:], Alu.mult)
            nc.vector.tensor_scalar(out=bvnv[:], in0=bvnv[:], scalar1=-1.0,
                                    scalar2=None, op0=Alu.mult)
            nc.vector.tensor_tensor(bvnu[:], zeffu[:], sut_sb[:], Alu.mult)
            nc.vector.tensor_scalar(out=bvnu[:], in0=bvnu[:], scalar1=-1.0,
                                    scalar2=None, op0=Alu.mult)
            for t in range(4, OT):
                nibus.append(u_unpack(t))

            for t in range(OT):
                nibu = nibus[t]
                for g in range(GU):
                    col = t * GU + g
                    affine("u",
                           wut[:, t * RANK + g * 128: t * RANK + (g + 1) * 128],
                           nibu[:, g * 128:(g + 1) * 128],
                           zeffu[:, col:col + 1], sut_sb[:, col:col + 1],
                           bvnu[:, col:col + 1])

            # 4 transposes per [128, 4096] f16 psum tile at bank-aligned offsets
            # (1 KB banks for f16 -> offsets 1024 apart), then one strided copy.
            with tc.tile_pool(name="ups", bufs=2, space="PSUM") as ups:
                for t in range(OT):
                    for kq in range(2):
                        pt = ups.tile([128, 4096], F16, tag="up", name="up")
                        for kk in range(4):
                            k = kq * 4 + kk
                            nc.tensor.transpose(
                                pt[:, kk * 1024: kk * 1024 + 128],
                                wut[:, t * RANK + k * 128: t * RANK + (k + 1) * 128],
                                id16[:])
                        src = pt.rearrange("p (x c) -> p x c", x=4)[:, :, :128]
                        dsl = lhsT[:].rearrange("p (k o) -> p k o", k=KT)[
                            :, kq * 4:(kq + 1) * 4, t * 128:(t + 1) * 128]
                        copy_alt(dsl, src)

            # ---- V dequant + matmuls, interleaved ----
            def deq_rt(st, rt):
                rs = rhs[st]
                wps = STRIP // PACK
                nibv = nibp.tile([128, STRIP], I32, tag="nibv", name="nibv")
                nibv_r = nibv[:].rearrange("p (w j) -> p w j", j=PACK)
                words = qvt_sb[:, rt * (I_SL // PACK) + st * wps:
                               rt * (I_SL // PACK) + (st + 1) * wps]
                for j in range(PACK):
                    nc.vector.tensor_scalar(
                        out=nibv_r[:, :, j], in0=words, scalar1=4 * j, scalar2=15,
                        op0=Alu.logical_shift_right, op1=Alu.bitwise_and)
                for gs in range(STRIP // GROUPSIZE):
                    col = rt * GV + st * (STRIP // GROUPSIZE) + gs
                    affine("v",
                           rs[:, rt * STRIP + gs * 128: rt * STRIP + (gs + 1) * 128],
                           nibv[:, gs * 128:(gs + 1) * 128],
                           zeffv[:, col:col + 1], av[:, col:col + 1],
                           bvnv[:, col:col + 1])

            def mm_group(mps, st, h, m):
                pt = mps.tile([128, 512], F32, tag="mm", name="mmps")
                rs = rhs[st]
                for k in range(KT):
                    nc.tensor.matmul(
                        pt[:],
                        lhsT[:, k * O_SL + m * 128: k * O_SL + (m + 1) * 128],
                        rs[:, k * STRIP + h * 512: k * STRIP + (h + 1) * 512],
                        start=(k == 0), stop=(k == KT - 1))
                ot = outp.tile([128, 512], F32, tag="ot", name="ot")
                copy_alt(ot[:], pt[:])
                nc.sync.dma_start(
                    out=out[m * 128:(m + 1) * 128,
                            st * STRIP + h * 512: st * STRIP + (h + 1) * 512],
                    in_=ot[:])

            with tc.tile_pool(name="mps", bufs=8, space="PSUM") as mps:
                for rt in range(RT):
                    deq_rt(0, rt)
                for x in range(RT):
                    deq_rt(1, x)
                    mm_group(mps, 0, 0, x)
                for m in range(OT):
                    mm_group(mps, 0, 1, m)
                for h in range(STRIP // 512):
                    for m in range(OT):
                        mm_group(mps, 1, h, m)

    nc.compile()
    return nc


def _host_prep(qweight_V, qzeros_V, scales_V, qweight_U, qzeros_U, scales_U, S):
    """Layout-only host prep: slice/transpose packed words + tables into SBUF layouts."""
    in_maps = []
    for c in range(N_CORES):
        a, b = divmod(c, P_I)
        qv = qweight_V[b * (I_SL // PACK):(b + 1) * (I_SL // PACK), :]
        qvt_h = np.ascontiguousarray(
            qv.T.reshape(RT, 128, I_SL // PACK).transpose(1, 0, 2).reshape(128, -1))
        sv = scales_V.T[:, b * GV:(b + 1) * GV]
        svt_h = np.ascontiguousarray(
            sv.reshape(RT, 128, GV).transpose(1, 0, 2).reshape(128, -1))
        qzv_h = np.ascontiguousarray(qzeros_V[b * GV:(b + 1) * GV, :])
        qu = qweight_U[:, a * O_SL:(a + 1) * O_SL]
        qut_h = np.ascontiguousarray(
            qu.T.reshape(OT, 128, RANK // PACK).transpose(1, 0, 2).reshape(128, -1))
        su = scales_U.T[a * O_SL:(a + 1) * O_SL, :]
        sut_h = np.ascontiguousarray(
            su.reshape(OT, 128, GU).transpose(1, 0, 2).reshape(128, -1))
        qzu_h = np.ascontiguousarray(qzeros_U[:, a * (O_SL // PACK):(a + 1) * (O_SL // PACK)])
        s_h = np.ascontiguousarray(S.reshape(RT, 128).T)
        in_maps.append({
            "qvt": qvt_h, "svt": svt_h, "qzv": qzv_h,
            "qut": qut_h, "sut": sut_h, "qzu": qzu_h, "s": s_h,
        })
    return in_maps


def kernel(x, qweight_V, qzeros_V, scales_V, g_idx_V,
           qweight_U, qzeros_U, scales_U, g_idx_U, S, **_unused):
    global _NC_CACHE, LAST_RESULTS
    qweight_V = np.asarray(qweight_V, dtype=np.int32)
    qzeros_V = np.asarray(qzeros_V, dtype=np.int32)
    scales_V = np.asarray(scales_V, dtype=np.float32)
    qweight_U = np.asarray(qweight_U, dtype=np.int32)
    qzeros_U = np.asarray(qzeros_U, dtype=np.int32)
    scales_U = np.asarray(scales_U, dtype=np.float32)
    S = np.asarray(S, dtype=np.float32)

    if _NC_CACHE is None:
        _NC_CACHE = _build_nc()
    nc = _NC_CACHE

    in_maps = _host_prep(qweight_V, qzeros_V, scales_V,
                         qweight_U, qzeros_U, scales_U, S)
    res = run_bass_kernel_spmd(nc, in_maps, core_ids=list(range(N_CORES)), trace=TRACE)
    LAST_RESULTS = res

    O = np.empty((OUT_SIZE, IN_SIZE), dtype=np.float32)
    for c in range(N_CORES):
        a, b = divmod(c, P_I)
        O[a * O_SL:(a + 1) * O_SL, b * I_SL:(b + 1) * I_SL] = res.results[c]["out"]
    return O
